# revision 1
# baseline (speedup 1.0000x reference)
"""Trainium2 Bass kernel for CriticREM: 2-layer MLP (128->256->256) + alpha-combined
head stack, data-parallel over 8 NeuronCores.

Strategy:
  - Heads collapse on host (exactly as the reference does):
      Wc = sum_h alphas[h] * Wh[h],  bc = sum_h alphas[h] * bh[h].
  - Batch (65536) sharded 8 ways; weights replicated.
  - Activations kept feature-major on device ([features, batch]); the input
    transpose is done on host, laid out chunk-contiguous so every input DMA is
    one fully-sequential 256 KiB read.
  - All matmuls run in float32r (fp32 with 11-bit mantissa, fp32 PSUM
    accumulation) -> 1 cycle/row on the PE at N>=256, 4x faster than fp32.
    Inputs are pre-rounded on host; on-device relu outputs are written as
    float32r by the consuming engines.
  - head_mode "h2": the combined head runs with h2 as the stationary operand,
    producing batch-major [128,1] columns into a persistent PSUM accumulator;
    one transpose + copy + single wide DMA writes the whole output shard.
    head_mode "wc": classic Wc-stationary [1,CHUNK] head rows + per-chunk copy.
"""

import numpy as np

from concourse import bacc, tile
import concourse.mybir as mybir
from concourse.bass_utils import run_bass_kernel_spmd

B = 65536
STATE_DIM = 96
ACTION_DIM = 32
IN_F = STATE_DIM + ACTION_DIM  # 128
HID = 256
N_CORES = 8
BS = B // N_CORES              # 8192 rows per core
CHUNK = 512                    # moving-dim tile (max for 4-byte matmul)
N_CHUNK = BS // CHUNK          # 16
NSUB = CHUNK // 128            # 4 batch sub-tiles per chunk (h2 head)

F32 = mybir.dt.float32
F32R = mybir.dt.float32r
RELU = mybir.ActivationFunctionType.Relu
IDENT = mybir.ActivationFunctionType.Identity
ADD = mybir.AluOpType.add
MAX = mybir.AluOpType.max

DEFAULT_OPTS = dict(
    head_mode="wc",      # "h2" | "wc"
    obias="act",         # wc head: engine for per-chunk [1,CHUNK] copy
    x_bufs=4,
    h_bufs=2,
    z1_bufs=2,
    z2_bufs=1,
    split_dma=1,         # issue each x chunk as two half-DMAs on two queues
)


def _round_f32r(a: np.ndarray) -> np.ndarray:
    """fp32r = fp32 with mantissa truncated to 11 bits (what the PE consumes)."""
    b = np.ascontiguousarray(a, dtype=np.float32).view(np.uint32)
    return (b & np.uint32(0xFFFFF000)).view(np.float32)


_NC_CACHE = []


def _build(dyn_iters=None, opts=None):
    """Build the kernel module. dyn_iters=None -> normal single-pass kernel.
    dyn_iters=K wraps the whole compute body in a hardware For_i loop that
    repeats it K times over the same data (used only for timing: wall-clock
    slope vs K isolates on-device time from dispatch overhead)."""
    o = dict(DEFAULT_OPTS)
    if opts:
        o.update(opts)
    head_mode = o["head_mode"]

    nc = bacc.Bacc("TRN2", target_bir_lowering=False, debug=False)

    xT = nc.dram_tensor("xT", [N_CHUNK, IN_F, CHUNK], F32R, kind="ExternalInput")
    w1t = nc.dram_tensor("w1t", [IN_F, HID], F32R, kind="ExternalInput")
    w2t = nc.dram_tensor("w2t", [128, 2, HID], F32R, kind="ExternalInput")
    wc = nc.dram_tensor("wc", [128, 2], F32R, kind="ExternalInput")
    b1 = nc.dram_tensor("b1", [128, 2], F32, kind="ExternalInput")
    b2 = nc.dram_tensor("b2", [128, 2], F32, kind="ExternalInput")
    bc = nc.dram_tensor("bc", [128, 1], F32, kind="ExternalInput")
    if head_mode == "h2":
        ident = nc.dram_tensor("ident", [128, 128], F32, kind="ExternalInput")
        # fp32r ISA needs an even moving free-dim: Wc chunk duplicated into pairs
        wc2 = nc.dram_tensor("wc2", [128, 2, 2], F32R, kind="ExternalInput")
    out = nc.dram_tensor("out", [BS, 1], F32, kind="ExternalOutput")

    with tile.TileContext(nc) as tc:
        with (
            tc.tile_pool(name="w", bufs=1) as wp,
            tc.tile_pool(name="x", bufs=o["x_bufs"]) as xp,
            tc.tile_pool(name="h", bufs=o["h_bufs"]) as hp,
            tc.tile_pool(name="o", bufs=4) as op,
            tc.tile_pool(name="z1", bufs=o["z1_bufs"], space="PSUM") as z1p,
            tc.tile_pool(name="z2", bufs=o["z2_bufs"], space="PSUM") as z2p,
            tc.tile_pool(name="zo", bufs=(1 if head_mode == "h2" else 2),
                         space="PSUM") as zop,
        ):
            w1_sb = wp.tile([IN_F, HID], F32R, tag="w1")
            w2_sb = wp.tile([128, 2, HID], F32R, tag="w2")
            wc_sb = wp.tile([128, 2], F32R, tag="wc")
            b1_sb = wp.tile([128, 2], F32, tag="b1")
            b2_sb = wp.tile([128, 2], F32, tag="b2")
            bc_sb = wp.tile([128, 1], F32, tag="bc")
            # w1/b1 gate the first matmuls -> sync queue; the rest go on the
            # gpsimd queue so they don't delay the first x chunks
            nc.sync.dma_start(w1_sb[:], w1t.ap())
            nc.sync.dma_start(b1_sb[:], b1.ap())
            nc.gpsimd.dma_start(w2_sb[:], w2t.ap())
            nc.gpsimd.dma_start(wc_sb[:], wc.ap())
            nc.gpsimd.dma_start(b2_sb[:], b2.ap())
            nc.gpsimd.dma_start(bc_sb[:], bc.ap())
            if head_mode == "h2":
                id_sb = wp.tile([128, 128], F32, tag="ident")
                nc.sync.dma_start(id_sb[:], ident.ap())
                wc2_sb = wp.tile([128, 2, 2], F32R, tag="wc2")
                nc.sync.dma_start(wc2_sb[:], wc2.ap())

            def pass_body(zo_acc=None):
                for i in range(N_CHUNK):
                    x = xp.tile([IN_F, CHUNK], F32R, tag="x")
                    if o["split_dma"]:
                        half = CHUNK // 2
                        nc.sync.dma_start(x[:, 0:half], xT.ap()[i, :, 0:half])
                        nc.gpsimd.dma_start(x[:, half:CHUNK],
                                            xT.ap()[i, :, half:CHUNK])
                    else:
                        nc.sync.dma_start(x[:], xT.ap()[i])

                    # layer 1: z1 = W1 @ x  (feature-major), relu+bias on ACT
                    z1a = z1p.tile([128, CHUNK], F32, tag="z1a")
                    z1b = z1p.tile([128, CHUNK], F32, tag="z1b")
                    nc.tensor.matmul(z1a[:], w1_sb[:, 0:128], x[:])
                    nc.tensor.matmul(z1b[:], w1_sb[:, 128:256], x[:])
                    h1a = hp.tile([128, CHUNK], F32R, tag="h1a")
                    h1b = hp.tile([128, CHUNK], F32R, tag="h1b")
                    nc.scalar.activation(h1a[:], z1a[:], RELU, bias=b1_sb[:, 0:1])
                    nc.scalar.activation(h1b[:], z1b[:], RELU, bias=b1_sb[:, 1:2])

                    # layer 2: accumulate over both 128-wide k-chunks
                    z2a = z2p.tile([128, CHUNK], F32, tag="z2a")
                    z2b = z2p.tile([128, CHUNK], F32, tag="z2b")
                    nc.tensor.matmul(z2a[:], w2_sb[:, 0, 0:128], h1a[:], start=True, stop=False)
                    nc.tensor.matmul(z2a[:], w2_sb[:, 1, 0:128], h1b[:], start=False, stop=True)
                    nc.tensor.matmul(z2b[:], w2_sb[:, 0, 128:256], h1a[:], start=True, stop=False)
                    nc.tensor.matmul(z2b[:], w2_sb[:, 1, 128:256], h1b[:], start=False, stop=True)
                    h2a = hp.tile([128, CHUNK], F32R, tag="h2a")
                    h2b = hp.tile([128, CHUNK], F32R, tag="h2b")
                    nc.vector.tensor_scalar(h2a[:], z2a[:], b2_sb[:, 0:1], 0.0, ADD, MAX)
                    nc.vector.tensor_scalar(h2b[:], z2b[:], b2_sb[:, 1:2], 0.0, ADD, MAX)

                    if head_mode == "h2":
                        # batch-major head: out column-pair per 128-batch subtile
                        # (duplicated Wc column satisfies the even-N fp32r rule)
                        for jj in range(NSUB):
                            col = i * NSUB + jj
                            bsl = slice(jj * 128, (jj + 1) * 128)
                            nc.tensor.matmul(zo_acc[:, col, :], h2a[:, bsl],
                                             wc2_sb[:, 0, :], start=True, stop=False)
                            nc.tensor.matmul(zo_acc[:, col, :], h2b[:, bsl],
                                             wc2_sb[:, 1, :], start=False, stop=True)
                    else:
                        zo = zop.tile([1, CHUNK], F32, tag="zo")
                        nc.tensor.matmul(zo[:], wc_sb[:, 0:1], h2a[:], start=True, stop=False)
                        nc.tensor.matmul(zo[:], wc_sb[:, 1:2], h2b[:], start=False, stop=True)
                        ot = op.tile([1, CHUNK], F32, tag="o")
                        eng = o["obias"]
                        if eng == "alt":
                            eng = "act" if i % 2 == 0 else "dve"
                        if eng == "act":
                            nc.scalar.activation(ot[:], zo[:], IDENT,
                                                 bias=bc_sb[0:1, 0:1])
                        else:
                            nc.vector.tensor_scalar(ot[:], zo[:], bc_sb[0:1, 0:1],
                                                    None, ADD)
                        nc.sync.dma_start(
                            out.ap()[i * CHUNK:(i + 1) * CHUNK, 0:1], ot[:])

                if head_mode == "h2":
                    # epilogue: [128, 64(,2)] batch-major accumulator -> +bc ->
                    # transpose -> one wide DMA of the whole shard
                    zz = op.tile([128, N_CHUNK * NSUB], F32, tag="zz")
                    nc.vector.tensor_scalar(zz[:], zo_acc[:, :, 0], bc_sb[:, 0:1],
                                            None, ADD)
                    zt = zop.tile([N_CHUNK * NSUB, 128], F32, tag="zt")
                    nc.tensor.transpose(zt[:], zz[:], id_sb[:])
                    of = op.tile([N_CHUNK * NSUB, 128], F32, tag="of")
                    nc.vector.tensor_copy(of[:], zt[:])
                    nc.sync.dma_start(out.ap(), of[:])

            def make_zo():
                if head_mode != "h2":
                    return None
                return zop.tile([128, N_CHUNK * NSUB, 2], F32, tag="zoacc",
                                name="zoacc")

            if dyn_iters is None:
                pass_body(make_zo())
            else:
                with tc.For_i(0, dyn_iters, 1):
                    pass_body(make_zo())

    nc.compile()
    return nc


def _prep_maps(state, action, alphas, W1, b1, W2, b2, Wh, bh, head_mode="h2"):
    state = np.asarray(state, dtype=np.float32)
    action = np.asarray(action, dtype=np.float32)
    alphas = np.asarray(alphas, dtype=np.float32)
    W1 = np.asarray(W1, dtype=np.float32)
    b1 = np.asarray(b1, dtype=np.float32)
    W2 = np.asarray(W2, dtype=np.float32)
    b2 = np.asarray(b2, dtype=np.float32)
    Wh = np.asarray(Wh, dtype=np.float32)
    bh = np.asarray(bh, dtype=np.float32)

    # collapse the head stack exactly like the reference
    Wc = np.einsum("h,hod->od", alphas, Wh)[0]     # [256]
    bc = float(np.einsum("h,ho->o", alphas, bh)[0])

    w1t_np = _round_f32r(W1.T)                               # [128, 256]
    w2t_np = _round_f32r(W2.T.reshape(2, 128, HID).transpose(1, 0, 2))  # [128,2,256]
    wc_np = _round_f32r(Wc.reshape(2, 128).T)                # [128, 2]
    b1_np = np.ascontiguousarray(b1.reshape(2, 128).T)       # [128, 2]
    b2_np = np.ascontiguousarray(b2.reshape(2, 128).T)       # [128, 2]
    bc_np = np.full((128, 1), bc, dtype=np.float32)

    x = _round_f32r(np.concatenate([state, action], axis=1))  # [B, 128]

    in_maps = []
    for c in range(N_CORES):
        xs = x[c * BS:(c + 1) * BS]                          # [BS, 128]
        # chunk-contiguous feature-major layout: [N_CHUNK, 128, CHUNK]
        xs = np.ascontiguousarray(
            xs.reshape(N_CHUNK, CHUNK, IN_F).transpose(0, 2, 1))
        m = {"xT": xs, "w1t": w1t_np, "w2t": w2t_np, "wc": wc_np,
             "b1": b1_np, "b2": b2_np, "bc": bc_np}
        if head_mode == "h2":
            m["ident"] = np.eye(128, dtype=np.float32)
            m["wc2"] = np.ascontiguousarray(
                np.repeat(wc_np[:, :, None], 2, axis=2))   # [128, 2, 2]
        in_maps.append(m)
    return in_maps


def kernel(state, action, alphas, W1, b1, W2, b2, Wh, bh):
    if not _NC_CACHE:
        _NC_CACHE.append(_build())
    nc = _NC_CACHE[0]
    in_maps = _prep_maps(state, action, alphas, W1, b1, W2, b2, Wh, bh,
                         head_mode=DEFAULT_OPTS["head_mode"])
    res = run_bass_kernel_spmd(nc, in_maps, core_ids=list(range(N_CORES)))
    return np.concatenate([res.results[c]["out"] for c in range(N_CORES)], axis=0)



# revision 2
# speedup vs baseline: 37.4478x; 37.4478x over previous
"""Trainium2 Bass kernel for CriticREM: 2-layer MLP (128->256->256) + alpha-combined
head stack, data-parallel over 8 NeuronCores.

Strategy:
  - Heads collapse on host (exactly as the reference does):
      Wc = sum_h alphas[h] * Wh[h],  bc = sum_h alphas[h] * bh[h].
  - Batch (65536) sharded 8 ways; weights replicated.
  - Activations kept feature-major on device ([features, batch]); the input
    transpose is done on host, laid out chunk-contiguous so every input DMA is
    one fully-sequential 256 KiB read (split in half across two DMA queues).
  - All matmuls run in float32r (fp32 with 11-bit mantissa, fp32 PSUM
    accumulation) -> 1 cycle/row on the PE at N>=256, 4x faster than fp32.
    Inputs are pre-rounded on host; on-device relu outputs are written as
    float32r by the consuming engines.
  - Head ("rot" mode): the combined head runs Wc-stationary ([128,1] stationary
    -> ~free LDWEIGHTS) producing a [1, CHUNK] output row per chunk, but each
    chunk's output is routed to PE column-group (chunk%4) via tile_position, so
    four consecutive chunks land on partitions {0,32,64,96} of ONE PSUM bank.
    Every 4 chunks one strided [4, CHUNK] copy evacuates the bank to SBUF
    (512-cycle cost amortized over 4 chunks instead of per-chunk [1,512] ops)
    and one 8 KiB DMA stores rows [4q:4q+4] of the chunk-major [16, 512]
    output. Two zo banks alternate so evacuation overlaps the next quarter's
    accumulation. The scalar output bias bc is added on HOST (exact, free).
"""

import numpy as np

from concourse import bacc, tile
import concourse.mybir as mybir
from concourse.bass_utils import run_bass_kernel_spmd

B = 65536
STATE_DIM = 96
ACTION_DIM = 32
IN_F = STATE_DIM + ACTION_DIM  # 128
HID = 256
N_CORES = 8
BS = B // N_CORES              # 8192 rows per core
CHUNK = 512                    # moving-dim tile (max for 4-byte matmul)
N_CHUNK = BS // CHUNK          # 16
QROT = 4                       # chunks per head-output PSUM bank (col groups)

F32 = mybir.dt.float32
F32R = mybir.dt.float32r
RELU = mybir.ActivationFunctionType.Relu
IDENT = mybir.ActivationFunctionType.Identity
ADD = mybir.AluOpType.add
MAX = mybir.AluOpType.max

DEFAULT_OPTS = dict(
    x_bufs=4,
    h_bufs=2,
    z1_bufs=2,
    z2_bufs=1,
    evac="alt",          # engine for quarter evacuation: "dve" | "act" | "alt"
)


def _round_f32r(a: np.ndarray) -> np.ndarray:
    """fp32r = fp32 with mantissa truncated to 11 bits (what the PE consumes)."""
    b = np.ascontiguousarray(a, dtype=np.float32).view(np.uint32)
    return (b & np.uint32(0xFFFFF000)).view(np.float32)


_NC_CACHE = []


def _build(dyn_iters=None, opts=None):
    """Build the kernel module. dyn_iters=None -> normal single-pass kernel.
    dyn_iters=K wraps the whole compute body in a hardware For_i loop that
    repeats it K times over the same data (used only for timing: wall-clock
    slope vs K isolates on-device time from dispatch overhead)."""
    o = dict(DEFAULT_OPTS)
    if opts:
        o.update(opts)

    nc = bacc.Bacc("TRN2", target_bir_lowering=False, debug=False)

    xT = nc.dram_tensor("xT", [N_CHUNK, IN_F, CHUNK], F32R, kind="ExternalInput")
    w1t = nc.dram_tensor("w1t", [IN_F, HID], F32R, kind="ExternalInput")
    w2t = nc.dram_tensor("w2t", [128, 2, HID], F32R, kind="ExternalInput")
    wc = nc.dram_tensor("wc", [128, 2], F32R, kind="ExternalInput")
    b1 = nc.dram_tensor("b1", [128, 2], F32, kind="ExternalInput")
    b2 = nc.dram_tensor("b2", [128, 2], F32, kind="ExternalInput")
    out = nc.dram_tensor("out", [N_CHUNK, CHUNK], F32, kind="ExternalOutput")

    with tile.TileContext(nc) as tc:
        with (
            tc.tile_pool(name="w", bufs=1) as wp,
            tc.tile_pool(name="x", bufs=o["x_bufs"]) as xp,
            tc.tile_pool(name="h", bufs=o["h_bufs"]) as hp,
            tc.tile_pool(name="o", bufs=2) as op,
            tc.tile_pool(name="z1", bufs=o["z1_bufs"], space="PSUM") as z1p,
            tc.tile_pool(name="z2", bufs=o["z2_bufs"], space="PSUM") as z2p,
            tc.tile_pool(name="zo", bufs=2, space="PSUM") as zop,
        ):
            w1_sb = wp.tile([IN_F, HID], F32R, tag="w1")
            w2_sb = wp.tile([128, 2, HID], F32R, tag="w2")
            wc_sb = wp.tile([128, 2], F32R, tag="wc")
            b1_sb = wp.tile([128, 2], F32, tag="b1")
            b2_sb = wp.tile([128, 2], F32, tag="b2")
            # startup order matters: only w1/b1 gate the first matmuls -> they
            # lead the sync queue, then chunk DMAs follow immediately. The
            # remaining weights ride the gpsimd queue behind chunk 0's second
            # half, ordered by first use (wc needed right after first L2).
            nc.sync.dma_start(w1_sb[:], w1t.ap())
            nc.sync.dma_start(b1_sb[:], b1.ap())

            def pass_body():
                zoq = None
                for i in range(N_CHUNK):
                    g = i % QROT
                    q = i // QROT
                    x = xp.tile([IN_F, CHUNK], F32R, tag="x")
                    half = CHUNK // 2
                    nc.sync.dma_start(x[:, 0:half], xT.ap()[i, :, 0:half])
                    nc.gpsimd.dma_start(x[:, half:CHUNK], xT.ap()[i, :, half:CHUNK])
                    if i == 0:
                        # late weights: queued on gpsimd after x0's second half
                        nc.gpsimd.dma_start(wc_sb[:], wc.ap())
                        nc.gpsimd.dma_start(w2_sb[:], w2t.ap())
                        nc.gpsimd.dma_start(b2_sb[:], b2.ap())

                    # layer 1: z1 = W1 @ x  (feature-major), relu+bias on ACT
                    z1a = z1p.tile([128, CHUNK], F32, tag="z1a")
                    z1b = z1p.tile([128, CHUNK], F32, tag="z1b")
                    nc.tensor.matmul(z1a[:], w1_sb[:, 0:128], x[:])
                    nc.tensor.matmul(z1b[:], w1_sb[:, 128:256], x[:])
                    h1a = hp.tile([128, CHUNK], F32R, tag="h1a")
                    h1b = hp.tile([128, CHUNK], F32R, tag="h1b")
                    nc.scalar.activation(h1a[:], z1a[:], RELU, bias=b1_sb[:, 0:1])
                    nc.scalar.activation(h1b[:], z1b[:], RELU, bias=b1_sb[:, 1:2])

                    # layer 2: accumulate over both 128-wide k-chunks
                    z2a = z2p.tile([128, CHUNK], F32, tag="z2a")
                    z2b = z2p.tile([128, CHUNK], F32, tag="z2b")
                    nc.tensor.matmul(z2a[:], w2_sb[:, 0, 0:128], h1a[:], start=True, stop=False)
                    nc.tensor.matmul(z2a[:], w2_sb[:, 1, 0:128], h1b[:], start=False, stop=True)
                    nc.tensor.matmul(z2b[:], w2_sb[:, 0, 128:256], h1a[:], start=True, stop=False)
                    nc.tensor.matmul(z2b[:], w2_sb[:, 1, 128:256], h1b[:], start=False, stop=True)
                    h2a = hp.tile([128, CHUNK], F32R, tag="h2a")
                    h2b = hp.tile([128, CHUNK], F32R, tag="h2b")
                    nc.vector.tensor_scalar(h2a[:], z2a[:], b2_sb[:, 0:1], 0.0, ADD, MAX)
                    nc.vector.tensor_scalar(h2b[:], z2b[:], b2_sb[:, 1:2], 0.0, ADD, MAX)

                    # head: Wc stationary ([128,1] -> ~free weight load); route
                    # chunk i's [1, CHUNK] output to col-group g = i%4 so one
                    # PSUM bank collects 4 chunks on partitions {0,32,64,96}
                    if g == 0:
                        zoq = zop.tile([128, CHUNK], F32, tag="zo")
                    zrow = zoq[32 * g:32 * g + 1, :]
                    nc.tensor.matmul(zrow, wc_sb[:, 0:1], h2a[:],
                                     start=True, stop=False, tile_position=(0, 32 * g))
                    nc.tensor.matmul(zrow, wc_sb[:, 1:2], h2b[:],
                                     start=False, stop=True, tile_position=(0, 32 * g))

                    if g == QROT - 1:
                        # evacuate the full quarter: strided [4, CHUNK] copy
                        # (bc is added on host), then one 8 KiB store
                        oq = op.tile([QROT, CHUNK], F32, tag="oq")
                        eng = o["evac"]
                        if eng == "alt":
                            eng = "dve" if q % 2 == 0 else "act"
                        if eng == "act":
                            nc.scalar.activation(oq[:], zoq[0:97:32, :], IDENT)
                        else:
                            nc.vector.tensor_copy(oq[:], zoq[0:97:32, :])
                        nc.sync.dma_start(out.ap()[QROT * q:QROT * (q + 1), :], oq[:])

            if dyn_iters is None:
                pass_body()
            else:
                with tc.For_i(0, dyn_iters, 1):
                    pass_body()

    nc.compile()
    return nc


def _prep_maps(state, action, alphas, W1, b1, W2, b2, Wh, bh):
    state = np.asarray(state, dtype=np.float32)
    action = np.asarray(action, dtype=np.float32)
    alphas = np.asarray(alphas, dtype=np.float32)
    W1 = np.asarray(W1, dtype=np.float32)
    b1 = np.asarray(b1, dtype=np.float32)
    W2 = np.asarray(W2, dtype=np.float32)
    b2 = np.asarray(b2, dtype=np.float32)
    Wh = np.asarray(Wh, dtype=np.float32)
    bh = np.asarray(bh, dtype=np.float32)

    # collapse the head stack exactly like the reference
    Wc = np.einsum("h,hod->od", alphas, Wh)[0]     # [256]
    bc = float(np.einsum("h,ho->o", alphas, bh)[0])

    w1t_np = _round_f32r(W1.T)                               # [128, 256]
    w2t_np = _round_f32r(W2.T.reshape(2, 128, HID).transpose(1, 0, 2))  # [128,2,256]
    wc_np = _round_f32r(Wc.reshape(2, 128).T)                # [128, 2]
    b1_np = np.ascontiguousarray(b1.reshape(2, 128).T)       # [128, 2]
    b2_np = np.ascontiguousarray(b2.reshape(2, 128).T)       # [128, 2]

    x = _round_f32r(np.concatenate([state, action], axis=1))  # [B, 128]

    in_maps = []
    for c in range(N_CORES):
        xs = x[c * BS:(c + 1) * BS]                          # [BS, 128]
        # chunk-contiguous feature-major layout: [N_CHUNK, 128, CHUNK]
        xs = np.ascontiguousarray(
            xs.reshape(N_CHUNK, CHUNK, IN_F).transpose(0, 2, 1))
        m = {"xT": xs, "w1t": w1t_np, "w2t": w2t_np, "wc": wc_np,
             "b1": b1_np, "b2": b2_np}
        in_maps.append(m)
    return in_maps, bc


def kernel(state, action, alphas, W1, b1, W2, b2, Wh, bh):
    if not _NC_CACHE:
        _NC_CACHE.append(_build())
    nc = _NC_CACHE[0]
    in_maps, bc = _prep_maps(state, action, alphas, W1, b1, W2, b2, Wh, bh)
    res = run_bass_kernel_spmd(nc, in_maps, core_ids=list(range(N_CORES)))
    out = np.concatenate(
        [res.results[c]["out"].reshape(BS, 1) for c in range(N_CORES)], axis=0)
    return out + np.float32(bc)
